# revision 35
# baseline (speedup 1.0000x reference)
"""
Multi-head attention (B=2, S=2048, D=1024, H=16, hd=64) on 8 TRN2 NeuronCores.

Sharding: tensor-parallel over (batch, head-group).
  core = b*4 + g   (b in {0,1}, g in {0..3})  owns batch b, heads 4g..4g+3.

v4: v3 + fp8 DoubleRow ctx + DMA queue spreading + drain engine split.
  - scores per j-tile: two K=64 matmuls row-packed on PE row groups 0-1 /
    2-3 (concurrent), psum [128, 1024] double-buffered so scores of j-tile
    t+1 run while exp of j-tile t is in flight.
  - exp alternates between ScalarE (ACT Exp) and a custom single-uop DVE
    polynomial: exp(x/8) ~= (((c3*x+c2)*x+c1)*x+c0)^4, valid |x|<=21.8.
    exp output is fp8e4 (e4m3, max 240; e <= e^2.7 ~ 15) written into
    j-pair tiles e[p, head, jtile, q] so ctx can run fp8 DoubleRow.
  - ctx via fp8 DoubleRow matmuls (0.5 cyc/row, 2 j-tiles per pass):
    per (j-pair, head): one M=64 v matmul into psum rows 0:63 + one M=1
    mask-column matmul into psum row 64 (denominator) — psum layout
    [65, 512] identical to v3 so drains/normalize/proj are unchanged.
  - V projection computed once (N=256 matmuls), drained to fp8 j-pair
    interleaved layout vext[p, pair, head, jtile, d].
  - normalize + output projection PER CHUNK, emitted one chunk late so they
    overlap the next chunk's attention; only the last chunk's tail is
    exposed. Partial outputs per head-pair in bf16, summed on the host.
  - input DMAs spread across sync/scalar/vector/gpsimd queues, first
    emit_qk interleaved with its own slice DMAs (progressive per-queue
    semaphore deps); out DMAs alternate gpsimd/sync.
  - ctx psum drain copies on gpsimd (Pool), recips on DVE, in parallel.
PSUM: sc 2x2 banks + ctx 2 banks + shared qkv/normalize/proj pool 2 = 8.
"""

import ml_dtypes
import numpy as np

BF16 = ml_dtypes.bfloat16

B, S, D = 2, 2048, 1024
H, HD = 16, 64
NCORES = 8
KSLICES = D // 128  # 8
QCHUNK = 512
NQC = S // QCHUNK  # 4
JT = S // 128  # 16 j tiles
JP = JT // 2  # 8 j pairs
RT = S // 128  # 16 row tiles
VW = HD + 1  # 65: psum rows = v columns + denominator row

# exp(x/8) ~= poly(x)^4 coefficients (near-minimax on |x/32|<=0.685)
EXP_C0 = 0.99904327235933443
EXP_C1 = 0.031326658265914201
EXP_C2 = 0.00050672396170171354
EXP_C3 = 4.967545531334575e-06

_cache = {}
DVE_EXP_DISABLE = False


def _patch_ldw_opt():
    # opt-in (KBENCH_LDW_OPT=1): flip walrus --enable-ldw-opt to true
    import os

    if os.environ.get("KBENCH_LDW_OPT") != "1":
        return
    from concourse import bass_utils

    if getattr(bass_utils, "_ldw_patched", False):
        return
    orig = bass_utils.run_command

    def run_command(cmd, *a, **k):
        cmd = [
            c.replace("--enable-ldw-opt=false", "--enable-ldw-opt=true")
            if isinstance(c, str)
            else c
            for c in cmd
        ]
        return orig(cmd, *a, **k)

    bass_utils.run_command = run_command
    bass_utils._ldw_patched = True


_patch_ldw_opt()


def _register_exp_op():
    import concourse.dve_ops as dve_ops
    from concourse.dve_spec import Spec, Src0, Src1, C0, C1, C2, sq

    if "EXP8_POLY_ANT" in dve_ops._SUB_OPCODE_FOR_NAME:
        return dve_ops._EXP8_POLY_ANT  # already registered in this process

    def _ref(in0, in1, s0, s1, imm2):
        x = in0.astype(np.float32)
        p = ((in1 * x + s0) * x + s1) * x + imm2
        return (p * p) * (p * p)

    op = dve_ops.DveOp(
        "EXP8_POLY_ANT",
        Spec(
            body=sq(sq(((Src1 * Src0 + C0) * Src0 + C1) * Src0 + C2)),
            reference=_ref,
        ),
        subdim=False,
        uops_sha={"v3": "9e8248c43016c357", "v4": "5f7d5757095f4782"},
    )
    dve_ops.OPS.append(op)
    dve_ops.CUSTOM_DVE_SPECS[op.name] = op.spec
    dve_ops._SUB_OPCODE_FOR_NAME[op.name] = (
        max(dve_ops._SUB_OPCODE_FOR_NAME.values()) + 1
    )
    dve_ops._EXP8_POLY_ANT = op
    return op


def _build_program():
    import concourse.bass as bass
    import concourse.tile as tile
    from concourse import bacc, mybir

    exp_op = _register_exp_op()

    f32 = mybir.dt.float32
    bf16 = mybir.dt.bfloat16
    fp8 = mybir.dt.float8e4
    Exp = mybir.ActivationFunctionType.Exp
    DR = mybir.MatmulPerfMode.DoubleRow

    nc = bacc.Bacc(
        "TRN2",
        target_bir_lowering=False,
        debug=False,
        num_devices=NCORES,
        enable_partition_id=False,
    )

    xT_d = nc.dram_tensor("xT", [D, S], bf16, kind="ExternalInput").ap()
    wqk_d = nc.dram_tensor("wqk", [D, 512], bf16, kind="ExternalInput").ap()
    bqk_d = nc.dram_tensor("bqk", [128, 4], f32, kind="ExternalInput").ap()
    wv_d = nc.dram_tensor("wv", [D, 256], bf16, kind="ExternalInput").ap()
    wp_d = nc.dram_tensor("wp", [256, D], bf16, kind="ExternalInput").ap()
    maskf_d = nc.dram_tensor("maskf", [128, RT], f32, kind="ExternalInput").ap()
    sel_d = nc.dram_tensor("sel", [2, 128], bf16, kind="ExternalInput").ap()
    out_ds = [
        nc.dram_tensor(f"out{p}", [S, D], bf16, kind="ExternalOutput").ap()
        for p in range(2)
    ]

    def mm(out, lhsT, rhs, **kw):
        nc.tensor.matmul(out, lhsT, rhs, **kw)

    with tile.TileContext(nc) as tc:
        with tc.tile_pool(name="persist", bufs=1) as pp:
            qkT = pp.tile([128, 4 * S], bf16, tag="qkT")
            # fp8 v, j-pair interleaved: col = pair*512 + head*128 + jhalf*64 + d
            vext = pp.tile([128, JP * 512], fp8, tag="vext")
            wp_sb = pp.tile([128, 2 * D], bf16, tag="wp")
            maskf = pp.tile([128, RT], f32, tag="maskf")
            # mask as fp8 head-diagonal DoubleRow weights, one [2,2] diag per
            # j-tile t: plane0 cols 2t:2t+2 = [m_t, 0], plane1 = [0, m_t]
            # (pack stride 32B per ISA lw alignment; diag selects the head)
            maskd8 = pp.tile([128, 64], fp8, tag="maskd8")
            bqk = pp.tile([128, 4], f32, tag="bqk")
            c3bc = pp.tile([128, 2 * QCHUNK], f32, tag="c3bc")
            ctxT = pp.tile([128, 2 * S], bf16, tag="ctxT")
            # norm broadcast selectors: sel[:, 0:64] = [1;0], [:, 64:128] = [0;1]
            sel = pp.tile([2, 128], bf16, tag="sel")
            # softmax 1/denominators: row 0 = head A, row 1 = head B
            rcp = pp.tile([2, QCHUNK], bf16, tag="rcp")
            xT = pp.tile([128, KSLICES * S], bf16, tag="xT")
            wqk = pp.tile([128, KSLICES * 512], bf16, tag="wqk")
            wv = pp.tile([128, KSLICES * 256], bf16, tag="wv")

            nc.gpsimd.dma_start(bqk[:], bqk_d[:])
            nc.gpsimd.dma_start(maskf[:], maskf_d[:])
            nc.gpsimd.dma_start(sel[:], sel_d[:])
            nc.gpsimd.memset(c3bc[:], EXP_C3)
            nc.gpsimd.memset(maskd8[:], 0.0)
            md4 = maskd8[:].rearrange("p (pl t two) -> p pl t two", pl=2, two=2)
            nc.vector.tensor_copy(md4[:, 0, :, 0], maskf[:])
            nc.vector.tensor_copy(md4[:, 1, :, 1], maskf[:])

            with (
                tc.tile_pool(name="pj", bufs=1, space="PSUM") as pj,
                tc.tile_pool(name="sc", bufs=2, space="PSUM") as scp,
                tc.tile_pool(name="cx", bufs=2, space="PSUM") as cxp,
                tc.tile_pool(name="dn", bufs=1, space="PSUM") as dnp,
                tc.tile_pool(name="ep", bufs=3) as ep,
                tc.tile_pool(name="ob", bufs=4) as ob,
            ):
                # ---------- qkv projections ----------
                # pj is a single bank; prefix work ping-pongs pj with the
                # denominator bank (dnp, free until attention starts)
                def psum512(pool):
                    # borrow an sc-tagged buffer (idle outside attention)
                    # when asked, else the single pj bank
                    if pool is scp:
                        t = scp.tile(
                            [128, 2 * QCHUNK], f32, tag="sc", name="scb"
                        )
                        return t[:, 0:QCHUNK]
                    return pj.tile([128, QCHUNK], f32, tag="pj", name="pjb")

                def emit_qk(pt, c, pool=None):
                    ps = psum512(pool)
                    for k in range(KSLICES):
                        mm(
                            ps[:],
                            wqk[:, k * 512 + pt * 128 : k * 512 + (pt + 1) * 128],
                            xT[:, k * S + c * QCHUNK : k * S + (c + 1) * QCHUNK],
                            start=(k == 0),
                            stop=(k == KSLICES - 1),
                        )
                    nc.vector.tensor_scalar_add(
                        qkT[:, pt * S + c * QCHUNK : pt * S + (c + 1) * QCHUNK],
                        ps[:],
                        bqk[:, pt : pt + 1],
                    )

                def emit_qk_first():
                    # qT+kT p0 c0 carry their own DMAs interleaved so each
                    # matmul's per-queue semaphore dep stays progressive;
                    # kT accumulates in a borrowed sc bank (pj is 1-deep)
                    ps = pj.tile([128, QCHUNK], f32, tag="pj")
                    psk = psum512(scp)
                    for k in range(KSLICES):
                        nc.sync.dma_start(
                            wqk[:, k * 512 : (k + 1) * 512],
                            wqk_d[k * 128 : (k + 1) * 128, :],
                        )
                        nc.scalar.dma_start(
                            xT[:, k * S : k * S + QCHUNK],
                            xT_d[k * 128 : (k + 1) * 128, 0:QCHUNK],
                        )
                        for pt, dst in ((0, ps), (2, psk)):
                            mm(
                                dst[:],
                                wqk[:, k * 512 + pt * 128 : k * 512 + (pt + 1) * 128],
                                xT[:, k * S : k * S + QCHUNK],
                                start=(k == 0),
                                stop=(k == KSLICES - 1),
                            )
                    nc.vector.tensor_scalar_add(
                        qkT[:, 0:QCHUNK], ps[:], bqk[:, 0:1]
                    )
                    nc.vector.tensor_scalar_add(
                        qkT[:, 2 * S : 2 * S + QCHUNK], psk[:], bqk[:, 2:3]
                    )

                def emit_v(t, pool=None):
                    ps_full = psum512(pool)
                    ps = ps_full[:, 0:256]
                    for k in range(KSLICES):
                        mm(
                            ps,
                            xT[:, k * S + t * 128 : k * S + (t + 1) * 128],
                            wv[:, k * 256 : (k + 1) * 256],
                            start=(k == 0),
                            stop=(k == KSLICES - 1),
                        )
                    P, half = divmod(t, 2)
                    dst = vext[:, P * 512 : (P + 1) * 512].rearrange(
                        "p (h t d) -> p h t d", h=4, t=2
                    )[:, :, half, :]
                    nc.vector.tensor_scalar_mul(
                        dst,
                        ps.rearrange("p (h d) -> p h d", h=4),
                        maskf[:, t : t + 1],
                    )

                # ---------- attention ----------
                def emit_scores(p, c, jt, sc):
                    for half, (lo, hi) in enumerate(((0, 64), (64, 128))):
                        mm(
                            sc[:, half * QCHUNK : (half + 1) * QCHUNK],
                            qkT[
                                lo:hi,
                                (2 + p) * S + jt * 128 : (2 + p) * S + (jt + 1) * 128,
                            ],
                            qkT[lo:hi, p * S + c * QCHUNK : p * S + (c + 1) * QCHUNK],
                            start=True,
                            stop=True,
                        )

                # 9:7 ACT:DVE exp split (ACT is cheaper per tile; DVE also
                # carries drains) -- tuned from engine-busy traces
                ACT_JTS = {0, 2, 4, 5, 6, 8, 10, 12, 14, 15}

                def emit_exp(jt, sc, e):
                    half = jt % 2
                    src = sc[:].rearrange("p (h q) -> p h q", h=2)
                    dst = e[:].rearrange("p (h t q) -> p h t q", h=2, t=2)[
                        :, :, half, :
                    ]
                    if jt in ACT_JTS or DVE_EXP_DISABLE:
                        nc.scalar.activation(dst, src, Exp, scale=0.125)
                    else:
                        nc.vector._custom_dve(
                            exp_op,
                            out=dst,
                            in0=src,
                            in1=c3bc[:],  # 1D-free src1 keeps the imm2 slot
                            s0=EXP_C2,
                            s1=EXP_C1,
                            imm2=EXP_C0,
                        )

                def emit_ctx_pair(p, P, e, ctxA, ctxB, den):
                    ev = e[:].rearrange("p (h t q) -> p h t q", h=2, t=2)
                    for half, ctx_ps in ((0, ctxA), (1, ctxB)):
                        h = 2 * p + half
                        vT = vext[
                            :, P * 512 + h * 128 : P * 512 + (h + 1) * 128
                        ].rearrange("p (t d) -> p t d", t=2)
                        mm(
                            ctx_ps[:],
                            vT,
                            ev[:, half],
                            start=(P == 0),
                            stop=(P == JP - 1),
                            perf_mode=DR,
                            skip_group_check=True,
                        )
                    # denominators for both heads at once: DoubleRow pack =
                    # heads, [2,2] per-partition mask diag routes head A -> row
                    # 0, head B -> row 1 (dual-fp8 dst must start at part. 0)
                    md = maskd8[:].rearrange(
                        "p (pl t two) -> p pl (t two)", pl=2, two=2
                    )
                    for t in range(2):
                        mm(
                            den[:],
                            md[:, :, 2 * (2 * P + t) : 2 * (2 * P + t) + 2],
                            e[:].rearrange("p (h tq) -> p h tq", h=2)[
                                :, :, t * 512 : (t + 1) * 512
                            ],
                            start=(P == 0 and t == 0),
                            stop=(P == JP - 1 and t == 1),
                            perf_mode=DR,
                            skip_group_check=True,
                        )

                def emit_drains(p, c, ctxA, ctxB, den):
                    from concourse.dve_ops import (
                        RECIP_APPROX_FAST_CONSTS as _RC,
                        RECIPROCAL_APPROX_FAST as _RA,
                    )

                    # recip first (frees the single denominator bank for the
                    # next chunk); bf16 out so the broadcast matmul runs at
                    # full bf16 PE rate. ctx copies on ACT run in parallel.
                    nc.vector._custom_dve(
                        _RA,
                        out=rcp[:],
                        in0=den[:],
                        s0=_RC["s0"],
                        s1=_RC["s1"],
                        imm2=_RC["imm2"],
                    )
                    for ctx_ps, h in ((ctxA, 2 * p), (ctxB, 2 * p + 1)):
                        half = h % 2
                        nc.scalar.copy(
                            ctxT[
                                half * HD : (half + 1) * HD,
                                p * S + c * QCHUNK : p * S + (c + 1) * QCHUNK,
                            ],
                            ctx_ps[:],
                        )

                def emit_chunk(p, c, extra=()):
                    ctxA = cxp.tile([HD, QCHUNK], f32, tag="ctx")
                    ctxB = cxp.tile([HD, QCHUNK], f32, tag="ctx")
                    den = dnp.tile([2, QCHUNK], f32, tag="den")
                    pend = None
                    e = None
                    extra = list(extra)
                    for jt in range(JT):
                        sc = scp.tile([128, 2 * QCHUNK], f32, tag="sc")
                        emit_scores(p, c, jt, sc)
                        if jt % 2 == 0:
                            e = ep.tile([128, 4 * QCHUNK], fp8, tag="e")
                        emit_exp(jt, sc, e)
                        if jt % 2 == 1:
                            if pend is not None:
                                emit_ctx_pair(p, pend[0], pend[1], ctxA, ctxB, den)
                            pend = (jt // 2, e)
                        if jt >= 2 and extra:
                            extra.pop(0)()
                    emit_ctx_pair(p, pend[0], pend[1], ctxA, ctxB, den)
                    for ex in extra:
                        ex()
                    emit_drains(p, c, ctxA, ctxB, den)

                def emit_normalize_chunk(p, c):
                    for half, h in ((0, 2 * p), (1, 2 * p + 1)):
                        rb = pj.tile([128, QCHUNK], f32, tag="pj")
                        mm(
                            rb[0:HD, :],
                            sel[:, half * HD : (half + 1) * HD],
                            rcp[:],
                            start=True,
                            stop=True,
                        )
                        sl = ctxT[
                            half * HD : (half + 1) * HD,
                            p * S + c * QCHUNK : p * S + (c + 1) * QCHUNK,
                        ]
                        nc.vector.tensor_mul(sl, sl, rb[0:HD, :])

                def emit_proj_oc(p, qt, oc, o, pool=None):
                    ps = psum512(pool)
                    mm(
                        ps[:],
                        ctxT[:, p * S + qt * 128 : p * S + (qt + 1) * 128],
                        wp_sb[:, p * D + oc * QCHUNK : p * D + (oc + 1) * QCHUNK],
                        start=True,
                        stop=True,
                    )
                    dst = o[:, oc * QCHUNK : (oc + 1) * QCHUNK]
                    if oc == 0:
                        nc.scalar.copy(dst, ps[:])
                    else:
                        nc.vector.tensor_copy(dst, ps[:])
                    if oc == 1:
                        eng = nc.gpsimd if qt % 2 == 0 else nc.sync
                        eng.dma_start(
                            out_ds[p][qt * 128 : (qt + 1) * 128, :], o[:]
                        )

                def tail_closures(p, c):
                    """normalize chunk (p,c) + its 4 proj pieces, split per
                    output half so the single pj bank never back-to-backs."""
                    cl = [lambda p=p, c=c: emit_normalize_chunk(p, c)]
                    for qt in range(4 * c, 4 * c + 4):
                        o = ob.tile([128, D], bf16, tag="o")
                        for oc in range(2):
                            cl.append(
                                lambda p=p, qt=qt, oc=oc, o=o: emit_proj_oc(
                                    p, qt, oc, o
                                )
                            )
                    return cl

                # minimal prefix for attention (p0, c0): qT p0 c0, kT p0, V.
                # first emit_qk carries its own DMAs; then queue the
                # remaining input DMAs spread across engines' DGE queues.
                emit_qk_first()
                for k in range(KSLICES):
                    nc.scalar.dma_start(
                        xT[:, k * S + QCHUNK : k * S + 2 * QCHUNK],
                        xT_d[k * 128 : (k + 1) * 128, QCHUNK : 2 * QCHUNK],
                    )
                for k in range(KSLICES):
                    nc.sync.dma_start(
                        xT[:, k * S + 2 * QCHUNK : k * S + 3 * QCHUNK],
                        xT_d[k * 128 : (k + 1) * 128, 2 * QCHUNK : 3 * QCHUNK],
                    )
                    nc.gpsimd.dma_start(
                        wv[:, k * 256 : (k + 1) * 256],
                        wv_d[k * 128 : (k + 1) * 128, :],
                    )
                for k in range(KSLICES):
                    nc.gpsimd.dma_start(
                        xT[:, k * S + 3 * QCHUNK : k * S + 4 * QCHUNK],
                        xT_d[k * 128 : (k + 1) * 128, 3 * QCHUNK : 4 * QCHUNK],
                    )
                for p in range(2):
                    nc.sync.dma_start(
                        wp_sb[:, p * D : (p + 1) * D], wp_d[p * 128 : (p + 1) * 128, :]
                    )

                # consumers ordered by xT chunk arrival (~2.8us apart):
                # each chunk's V tiles, then its kT projection. The xT-c3
                # consumers ride chunk-0 slots instead - c3 lands after the
                # c2 work drains, so the in-order PE never parks on it
                for t in range(0, 4):
                    emit_v(t, pool=scp)
                emit_qk(2, 1, pool=scp)
                for t in range(4, 10):
                    emit_v(t, pool=scp)

                # leftover qkv work, interleaved into pair-0 attention chunks
                qkv_rest = [
                    [(2, 2), (2, 3), (0, 10), (0, 11), (0, 12), (0, 13),
                     (0, 14), (0, 15), (0, 1)],
                    [(0, 2), (1, 0), (3, 0)],
                    [(0, 3), (1, 1), (3, 1), (3, 2)],
                    [(1, 2), (1, 3), (3, 3)],
                ]

                pending = []  # tail closures from the previous chunk
                for p in range(2):
                    for c in range(NQC):
                        extras = []
                        if p == 0:
                            for pt, cc in qkv_rest[c]:
                                if pt == 0 and cc >= 10:
                                    extras.append(lambda t=cc: emit_v(t))
                                else:
                                    extras.append(
                                        lambda pt=pt, cc=cc: emit_qk(pt, cc)
                                    )
                        extras += list(pending)
                        pending = []
                        if (p, c) != (1, NQC - 1):
                            emit_chunk(p, c, extras)
                            pending = tail_closures(p, c)
                        else:
                            emit_chunk(p, c, extras)
                # exposed tail: last chunk's normalize + proj. Attention is
                # done, so ping-pong psum between the free sc pool and pj to
                # keep the pieces pipelined despite pj being a single bank.
                p, c = 1, NQC - 1
                emit_normalize_chunk(p, c)
                for i, qt in enumerate(range(4 * c, 4 * c + 4)):
                    o = ob.tile([128, D], bf16, tag="o")
                    emit_proj_oc(p, qt, 0, o, pool=scp)
                    emit_proj_oc(p, qt, 1, o, pool=scp)

    nc.compile()
    return nc


def get_program():
    if "nc" not in _cache:
        _cache["nc"] = _build_program()
    return _cache["nc"]


def make_in_maps(x, mask, W_qkv, b_qkv, W_proj):
    """Build the 8 per-core input maps (host-side sharding)."""
    x = np.asarray(x, dtype=np.float32)
    mask = np.asarray(mask)
    W_qkv = np.asarray(W_qkv, dtype=np.float32)
    b_qkv = np.asarray(b_qkv, dtype=np.float32)
    W_proj = np.asarray(W_proj, dtype=np.float32)

    in_maps = []
    for core in range(NCORES):
        b, g = divmod(core, 4)
        qc = slice(256 * g, 256 * (g + 1))  # q cols for heads 4g..4g+3
        kc = slice(D + 256 * g, D + 256 * (g + 1))

        xT = np.ascontiguousarray(x[b].T).astype(BF16)

        wqk = np.concatenate([W_qkv[:, qc], W_qkv[:, kc]], axis=1)
        wqk = np.ascontiguousarray(wqk).astype(BF16)

        bq = b_qkv[qc]
        bk = b_qkv[kc]
        bqk = np.stack([bq[:128], bq[128:], bk[:128], bk[128:]], axis=1)
        bqk = np.ascontiguousarray(bqk)

        wv = np.ascontiguousarray(
            W_qkv[:, 2 * D + 256 * g : 2 * D + 256 * (g + 1)]
        ).astype(BF16)

        wp = np.ascontiguousarray(W_proj[256 * g : 256 * (g + 1), :]).astype(BF16)

        maskf = np.ascontiguousarray(
            mask[b].astype(np.float32).reshape(RT, 128).T
        )  # [128, RT] col t = rowtile t

        in_maps.append(
            {
                "xT": xT,
                "wqk": wqk,
                "bqk": bqk,
                "wv": wv,
                "wp": wp,
                "maskf": maskf,
                "sel": np.kron(np.eye(2), np.ones((1, 64))).astype(BF16),
            }
        )
    return in_maps


def kernel(x, mask, W_qkv, b_qkv, W_proj, b_proj, _trace=False):
    from concourse import bass_utils

    nc = get_program()
    in_maps = make_in_maps(x, mask, W_qkv, b_qkv, W_proj)

    res = bass_utils.run_bass_kernel_spmd(
        nc, in_maps, list(range(NCORES)), trace=_trace
    )
    _cache["last_results"] = res

    b_qkv = np.asarray(b_qkv, dtype=np.float32)
    W_proj = np.asarray(W_proj, dtype=np.float32)
    bias_full = np.asarray(b_proj, dtype=np.float32) + b_qkv[2 * D :] @ W_proj

    out = np.empty((B, S, D), dtype=np.float32)
    for b in range(B):
        acc = bias_full[None, :].repeat(S, axis=0).astype(np.float32)
        for g in range(4):
            r = res.results[b * 4 + g]
            acc = acc + r["out0"].astype(np.float32) + r["out1"].astype(np.float32)
        out[b] = acc
    return out
